# revision 6
# baseline (speedup 1.0000x reference)
"""KNN palette-retrieval kernel for Trainium2 (8 NeuronCores, data-parallel).

Per pixel of rgb_mask [16,3,512,512]: find the palette row (of 21,
L2-normalized) with max cosine similarity, emit that normalized color;
zero pixels emit 0.  argmax(cos) == argmax(dot) since the pixel norm is a
positive scalar, so pixel normalization is skipped.

Layout: each core takes 2 batches = 524288 px, split into 32 "sets" g of
16384 px.  PE row layout = 32*k' + g (quadrant-aligned so every DVE
partition range starts at 0/32/64/96).  24 k-slots = 6 matmuls x 4 slots
(21 real + 3 zero-padded; padded sims are 0 and lose to the 1e-20 floor).

Per tile of 32x512 pixels:
  mm1_i (x6): pa_i[32k'+g, n] = sims for k=4i+k'   (PE, float32r: the PE's
          split-fp32 mode, 4x the fp32 matmul rate; ~1e-5 worse sims only
          move tie-break pixels, rel err 9.2e-3 vs the 2e-2 gate)
  chain:  s = copy(pa_0) (DVE);  4x running TT-max vs pa_1..pa_4 (DVE)
  fold:   max over quadrants (pa_5 as the PSUM operand lets the 64-row
          fold cross partition bases; then equal-base 32-row merges)
  mrep:   one f32 matmul with a 0/1 weight broadcasts m3[g] back to all
          128 rows (exact on the PE, so comparisons below stay exact)
  msb:    tensor_scalar_max(mrep, 1e-20)  (DVE, PSUM->SBUF; floors the
          all-zero-pixel case so padded slots never fire)
  oh_i:   TT is_ge(pa_i, msb) in {0,1}   (DVE; exact compare of the sims
          against their own max -> the argmax row always passes)
  mm3_i (x6): pout[32c+g] += cn[k,c]*oh_i  (PE float32r) -> palette color
  yout:   copy pout -> SBUF (ACT), DMA out.

vs the v1 kernel (6 extra f32 matmuls to subtract the max before a
tensor_scalar is_ge): PE time/tile drops ~15.4us -> ~3.4us and the whole
kernel runs at ~0.1ms/core, near the PE structural floor for this layout.
Input DMA rides qSp (sync), output DMA qAct (scalar) so the two HWDGE
queues work in parallel; I/O chunks are 4096 cols (1.5MB) for cheap
descriptors.

build_nc(reps=N) unrolls the whole pipeline N times inside one NEFF (same
weights, same I/O, idempotent) — used by test.py to measure true HW time
with a dispatch-floor-cancelling slope; kernel() always runs reps=1.
"""

import sys

sys.path.insert(0, "/opt/trn_rl_repo")

import numpy as np

B, C, H, W = 16, 3, 512, 512
K = 21
NCORES = 8
BPC = B // NCORES            # batches per core
PXC = BPC * H * W            # pixels per core = 524288
G = 32                       # pixel sets (partition-packed)
REG = PXC // G               # 16384 columns per set
NT = 512                     # pixel columns per compute tile
NTIO = 4096                  # pixel columns per DMA chunk
NMM = 6                      # k-slot matmuls (6*4 = 24 >= 21)

_CACHE: dict = {}


def build_nc(reps=1, mm_dt="float32r", mm3_dt="float32r", nt=NT,
             psum_bufs=1):
    key = ("nc", reps, mm_dt, mm3_dt, nt, psum_bufs)
    if key in _CACHE:
        return _CACHE[key]
    from contextlib import ExitStack

    import concourse.tile as tile
    from concourse import bacc, mybir

    f32 = mybir.dt.float32
    d1 = getattr(mybir.dt, mm_dt)    # sims matmul dtype
    d3 = getattr(mybir.dt, mm3_dt)   # color matmul dtype
    mx = mybir.AluOpType.max
    ge = mybir.AluOpType.is_ge

    nc = bacc.Bacc("TRN2", target_bir_lowering=False, debug=False,
                   num_devices=NCORES)
    x = nc.dram_tensor("x", [C * G, REG], d1, kind="ExternalInput").ap()
    w1 = nc.dram_tensor("w1", [NMM, C * G, 128], d1,
                        kind="ExternalInput").ap()
    w2 = nc.dram_tensor("w2", [G, 128], f32, kind="ExternalInput").ap()
    w3 = nc.dram_tensor("w3", [NMM, 128, C * G], d3,
                        kind="ExternalInput").ap()
    y = nc.dram_tensor("y", [C * G, REG], f32, kind="ExternalOutput").ap()

    with ExitStack() as ctx:
        tc = ctx.enter_context(tile.TileContext(nc))
        wp = ctx.enter_context(tc.tile_pool(name="w", bufs=1))
        inp = ctx.enter_context(tc.tile_pool(name="xin", bufs=2))
        sp = ctx.enter_context(tc.tile_pool(name="s", bufs=2))
        mp = ctx.enter_context(tc.tile_pool(name="m", bufs=2))
        ohp = ctx.enter_context(tc.tile_pool(name="oh", bufs=2))
        yp = ctx.enter_context(tc.tile_pool(name="y", bufs=2))
        pap = [ctx.enter_context(
            tc.tile_pool(name=f"pa{i}", bufs=psum_bufs, space="PSUM"))
            for i in range(NMM)]
        mrp = ctx.enter_context(
            tc.tile_pool(name="mr", bufs=psum_bufs, space="PSUM"))
        pop = ctx.enter_context(
            tc.tile_pool(name="po", bufs=psum_bufs, space="PSUM"))

        w1s, w3s = [], []
        for i in range(NMM):
            w1t = wp.tile([C * G, 128], d1, name=f"w1s{i}")
            nc.sync.dma_start(w1t[:], w1[i])
            w1s.append(w1t)
            w3t = wp.tile([128, C * G], d3, name=f"w3s{i}")
            nc.sync.dma_start(w3t[:], w3[i])
            w3s.append(w3t)
        w2s = wp.tile([G, 128], f32)
        nc.sync.dma_start(w2s[:], w2[:])

        for r in range(reps):
            for io in range(REG // NTIO):
                i0 = io * NTIO
                xin = inp.tile([C * G, NTIO], d1, tag="xin")
                nc.sync.dma_start(xin[:], x[:, i0:i0 + NTIO])
                yout = yp.tile([C * G, NTIO], f32, tag="yout")

                for t in range(NTIO // nt):
                    xs = xin[:, t * nt:(t + 1) * nt]

                    pa = []
                    for i in range(NMM):
                        pai = pap[i].tile([128, nt], f32, tag=f"pa{i}",
                                          name=f"pa{i}")
                        nc.tensor.matmul(pai[:], w1s[i][:], xs,
                                         start=True, stop=True)
                        pa.append(pai)

                    # running max over the 6 PSUM banks (<=1 PSUM operand
                    # per TT; SBUF+SBUF operands must share base partition)
                    s = sp.tile([128, nt], f32, tag="s")
                    sm = sp.tile([128, nt], f32, tag="sm")
                    nc.vector.tensor_copy(s[:], pa[0][:])
                    nc.vector.tensor_tensor(sm[:], pa[1][:], s[:], mx)
                    nc.vector.tensor_tensor(s[:], pa[2][:], sm[:], mx)
                    nc.vector.tensor_tensor(sm[:], pa[3][:], s[:], mx)
                    nc.vector.tensor_tensor(s[:], pa[4][:], sm[:], mx)
                    # fold quadrants: pa5 (1 real + 3 zero slots) as the
                    # PSUM operand lets the 64-row fold cross bases
                    u = sp.tile([64, nt], f32, tag="u")
                    nc.vector.tensor_tensor(u[:], pa[5][0:64, :],
                                            s[64:128, :], mx)
                    m1 = mp.tile([32, nt], f32, tag="m1")
                    m2 = mp.tile([32, nt], f32, tag="m2")
                    m3 = mp.tile([32, nt], f32, tag="m3")
                    nc.vector.tensor_tensor(m1[:], u[0:32, :], s[0:32, :], mx)
                    nc.vector.tensor_tensor(m2[:], u[32:64, :],
                                            s[32:64, :], mx)
                    nc.vector.tensor_tensor(m3[:], m1[:], m2[:], mx)

                    # broadcast m3 to all 128 rows: mrep[32k+g] = m3[g]
                    # (0/1 matmul, exact), then floor on the way to SBUF
                    mrep = mrp.tile([128, nt], f32, tag="mr", name="mr")
                    nc.tensor.matmul(mrep[:], w2s[:], m3[:],
                                     start=True, stop=True)
                    msb = sp.tile([128, nt], f32, tag="msb")
                    nc.vector.tensor_scalar_max(msb[:], mrep[:], 1e-20)

                    # all compares first, then the color matmuls: keeps DVE
                    # streaming instead of ping-ponging with the PE
                    pout = pop.tile([C * G, nt], f32, tag="po", name="po")
                    ohs = []
                    for i in range(NMM):
                        oh = ohp.tile([128, nt], d3, tag=f"oh{i}",
                                      name=f"oh{i}")
                        nc.vector.tensor_tensor(oh[:], pa[i][:], msb[:], ge)
                        ohs.append(oh)
                    for i in range(NMM):
                        nc.tensor.matmul(pout[:], w3s[i][:], ohs[i][:],
                                         start=(i == 0), stop=(i == NMM - 1))

                    nc.scalar.copy(yout[:, t * nt:(t + 1) * nt], pout[:])

                nc.scalar.dma_start(y[:, i0:i0 + NTIO], yout[:])

    nc.compile()
    _CACHE[key] = nc
    return nc


def _weights(colors: np.ndarray):
    cn = (colors.astype(np.float64)
          / np.linalg.norm(colors.astype(np.float64), axis=-1, keepdims=True))
    W1 = np.zeros((NMM, C * G, 128), np.float32)
    W2 = np.zeros((G, 128), np.float32)
    W3 = np.zeros((NMM, 128, C * G), np.float32)
    for i in range(NMM):
        for kp in range(4):
            k = 4 * i + kp
            if k >= K:
                continue
            for g in range(G):
                for c in range(C):
                    W1[i, G * c + g, G * kp + g] = cn[k, c]
                    W3[i, G * kp + g, G * c + g] = cn[k, c]
    for g in range(G):
        for kp in range(4):
            W2[g, G * kp + g] = 1.0
    return W1, W2, W3


def stage_inputs(rgb_mask: np.ndarray, colors: np.ndarray):
    W1, W2, W3 = _weights(np.asarray(colors, np.float32))
    in_maps = []
    for i in range(NCORES):
        xc = np.asarray(rgb_mask[BPC * i:BPC * (i + 1)], np.float32)
        xc = np.transpose(xc, (1, 0, 2, 3)).reshape(C * G, REG)
        in_maps.append({
            "x": np.ascontiguousarray(xc),
            "w1": W1, "w2": W2, "w3": W3,
        })
    return in_maps


def gather_outputs(results):
    outs = []
    for i in range(NCORES):
        yb = results[i]["y"].reshape(C, BPC, H, W)
        outs.append(np.transpose(yb, (1, 0, 2, 3)))
    return np.ascontiguousarray(np.concatenate(outs, axis=0))


def run(rgb_mask, colors, trace=False, **kw):
    from concourse.bass_utils import run_bass_kernel_spmd

    nc = build_nc()
    in_maps = stage_inputs(rgb_mask, colors)
    res = run_bass_kernel_spmd(nc, in_maps, core_ids=list(range(NCORES)),
                               trace=trace, **kw)
    return gather_outputs(res.results), res


def kernel(rgb_mask, colors):
    out, _ = run(rgb_mask, colors)
    return out


# back-compat aliases (v1 test harness used underscored names)
_build_nc = build_nc
_stage_inputs = stage_inputs
_gather_outputs = gather_outputs
